# revision 58
# baseline (speedup 1.0000x reference)
"""Per-pixel kernel-lookup conv for trn2, data-parallel over batch on 8 cores.

Per core (one image), per 512-pixel chunk: conv against all 128 kernels via
2 accumulating fp16 matmuls (host-side im2col split 72+72 rows: equal-K
pair avoids the PE's K-reconfiguration on the accumulating matmul),
DVE multiply by a host-precomputed fp8 one-hot mask straight out of PSUM,
ones-matmul partition-reduce lagged 2 chunks so the PE never waits on the
mask product, ACT pair-evacuation, inline output DMAs every 8 chunks.

Inputs stream as graduated per-tile DMAs (small first tiles so chunk 0
starts ~11us in) spread across the sync/scalar/gpsimd dispatchers, each
tile written by exactly one DMA so Tile's whole-tile dependency tracking
never over-serializes. Tails beyond the raster are zero-filled so every
chunk runs a uniform 512 columns; the one-hot mask is 0 on pad columns,
so garbage there never reaches the output.

Known ceilings (measured): a pure same-stationary matmul stream sustains
216ns/512-col MM; stationary rotation costs ~100ns/MM; a K=128+K=16
accumulating pair ran ~535ns/MM where the equal-K 72+72 pair runs
~426ns/MM median (the drain is partly a K-switch x accumulation
interaction). The DVE mask-multiply (~680ns/chunk, PSUM operand is
fp32-only) sits just above the full-clock PE chunk cadence. 1024-col
double-chunks, half-interleaving, no-accum restructures, deep lags, and
GPSIMD reduce were all tried and did not beat this configuration on HW.
"""
import numpy as np

RAST = 126 * 128  # output raster, 126 rows padded to 128 wide
NCH = (RAST + 511) // 512  # 32 chunks of <=512 cols
_NC_CACHE = {}


def _split_waits_json(bj: bytes) -> bytes:
    """Walrus rejects >4 sync-waits per instruction (and ~2 on Matmult).
    Split excess waits onto same-engine NoOps inserted just before."""
    import json

    j = json.loads(bj)
    ctr = 0
    for f in j["functions"]:
        for bb in f["blocks"]:
            out = []
            for inst in bb["instructions"]:
                si = inst.get("sync_info")
                cap = 1
                waits = (si or {}).get("on_wait") or []
                if len(waits) > cap:
                    extra, keep = waits[:-cap], waits[-cap:]
                    for g in range(0, len(extra), 1):
                        ctr += 1
                        out.append({
                            "debug": inst.get("debug", 0),
                            "engine": inst["engine"],
                            "ins": [],
                            "name": f"WS-{ctr}",
                            "opcode": "NoOp",
                            "outs": [],
                            "sync_info": {"on_update": [],
                                          "on_wait": extra[g:g + 1]},
                        })
                    si["on_wait"] = keep
                out.append(inst)
            bb["instructions"] = out
    return json.dumps(j).encode()


def _build_nc():
    from contextlib import ExitStack

    import concourse.bass as bass
    import concourse.tile as tile
    from concourse import mybir

    F32 = mybir.dt.float32
    F16 = mybir.dt.float16
    F8 = mybir.dt.float8e4
    import os
    PSDT = mybir.dt.bfloat16 if os.environ.get("PSC_BF16") else F32

    nc = bass.Bass(trn_type="TRN2", target_bir_lowering=False)
    bufA = nc.dram_tensor("bufA", [72, RAST], F16, kind="ExternalInput")
    bufB = nc.dram_tensor("bufB", [72, RAST], F16, kind="ExternalInput")
    oh = nc.dram_tensor("oh", [128, RAST], F8, kind="ExternalInput")
    w8 = nc.dram_tensor("w8", [72, 128], F16, kind="ExternalInput")
    w1 = nc.dram_tensor("w1", [72, 128], F16, kind="ExternalInput")
    o = nc.dram_tensor("o", [1, 16384], F32, kind="ExternalOutput")

    # SBUF input sub-tiles in 512-col multiples; each tile is written by
    # exactly one DMA so whole-tile dependency tracking never blocks early
    # chunks on late loads. Tails beyond RAST are zero-filled so every
    # chunk runs a uniform 512 cols.
    A_WS = [512] * 2 + [1024] * 15
    O_WS = [1024] * 2 + [2048] * 7
    B_WS = [512] * 2 + [1024] * 15

    with tile.TileContext(nc) as tc, ExitStack() as ctx:
        sb = ctx.enter_context(tc.tile_pool(name="sb", bufs=1))
        msk = ctx.enter_context(tc.tile_pool(name="msk", bufs=4))
        psc_pool = ctx.enter_context(tc.tile_pool(name="psc", bufs=4, space="PSUM"))
        pso_pool = ctx.enter_context(tc.tile_pool(name="pso", bufs=2, space="PSUM"))

        ones = sb.tile([128, 1], F16)
        nc.vector.memset(ones[:], 1.0)

        def load_tiles(widths, dram, parts, dt, engs, tag, split_first=0):
            # split_first: the first N tiles load as two partition-half
            # DMAs on parallel queues, halving their arrival latency so
            # the earliest chunks never wait (~4us of early gaps traced).
            tiles, c0 = [], 0
            for i, w in enumerate(widths):
                c1 = min(c0 + w, RAST)
                t = sb.tile([parts, w], dt, name=f"{tag}{i}")
                if c1 - c0 < w:
                    nc.vector.memset(t[:, c1 - c0:], 0.0)
                eng = engs[i % len(engs)]
                if i < split_first:
                    hp = parts // 2
                    eng.dma_start(t[:hp, :c1 - c0], dram.ap()[:hp, c0:c1])
                    eng.dma_start(t[hp:parts, :c1 - c0],
                                  dram.ap()[hp:parts, c0:c1])
                else:
                    eng.dma_start(t[:, :c1 - c0], dram.ap()[:, c0:c1])
                tiles.append((c0, w, t))
                c0 += w
            return tiles

        def pick(tiles, n0):
            for c0, w, t in tiles:
                if c0 <= n0 < c0 + w:
                    return t, n0 - c0
            raise AssertionError(n0)

        w8_t = sb.tile([72, 128], F16)
        nc.gpsimd.dma_start(w8_t[:], w8.ap())
        w1_t = sb.tile([72, 128], F16)
        nc.gpsimd.dma_start(w1_t[:], w1.ap())
        bufA_t = load_tiles(A_WS, bufA, 72, F16, [nc.sync], "bufA",
                            split_first=2)
        bufB_t = load_tiles(B_WS, bufB, 72, F16, [nc.gpsimd], "bufB",
                            split_first=2)
        oh_t = load_tiles(O_WS, oh, 128, F8, [nc.scalar], "oh",
                          split_first=2)

        out_sb = sb.tile([1, 16384], F32)

        # Two chunks per emission round, stationary-major: wa,wa,wb,wb,
        # ones,ones -- each stationary loads once per round (3 LDWs per 6
        # matmuls) and each accumulating wb_c is separated from its wa_c
        # by an independent matmul, hiding the PSUM RAW drain.
        m_l = [None] * NCH
        for base in range(0, NCH + 2, 2):
            group = []
            for h in range(2):
                c = base + h
                if c >= NCH:
                    continue
                n0 = c * 512
                ta, ao = pick(bufA_t, n0)
                tb, bo = pick(bufB_t, n0)
                to, oo = pick(oh_t, n0)
                psc = psc_pool.tile([128, 512], PSDT, name="psc")
                group.append((c, psc, ta, ao, tb, bo, to, oo))
            for c, psc, ta, ao, tb, bo, to, oo in group:
                nc.tensor.matmul(psc[:], lhsT=w8_t[:],
                                 rhs=ta[:, ao:ao + 512],
                                 start=True, stop=False)
            for c, psc, ta, ao, tb, bo, to, oo in group:
                nc.tensor.matmul(psc[:], lhsT=w1_t[:],
                                 rhs=tb[:, bo:bo + 512],
                                 start=False, stop=True)
            for c, psc, ta, ao, tb, bo, to, oo in group:
                m = msk.tile([128, 512], F16)
                m_l[c] = m
                nc.vector.tensor_tensor(
                    out=m[:], in0=to[:, oo:oo + 512],
                    in1=psc[:], op=mybir.AluOpType.mult)
            for h in range(2):
                r = base - 2 + h
                if r < 0 or r >= NCH:
                    continue
                if h == 0:
                    pso = pso_pool.tile([1, 1024], F32)
                nc.tensor.matmul(pso[:, h * 512:h * 512 + 512], lhsT=ones[:],
                                 rhs=m_l[r][:], start=True, stop=True)
                if h == 1:
                    g0 = (r - 1) * 512
                    nc.scalar.copy(out_sb[0:1, g0:g0 + 1024],
                                   pso[0:1, 0:1024])
                    if r % 8 == 7:
                        nc.scalar.dma_start(o.ap()[:, g0 - 3072:g0 + 1024],
                                            out_sb[0:1, g0 - 3072:g0 + 1024])

    orig = nc.to_json_bytes
    nc.to_json_bytes = lambda: _split_waits_json(orig())
    return nc


def _get_nc():
    if "nc" not in _NC_CACHE:
        _NC_CACHE["nc"] = _build_nc()
    return _NC_CACHE["nc"]


def _in_maps(data, kernel_idx, weights):
    import ml_dtypes

    B = data.shape[0]
    # w8[(dy*3+dx)*16+c, j] = weights[j, c, dy, dx] for taps 0..7; w1 tap 8
    wt = np.ascontiguousarray(
        np.transpose(weights, (2, 3, 1, 0)).reshape(144, 128)
    ).astype(np.float16)
    w8 = np.ascontiguousarray(wt[:72])
    w1 = np.ascontiguousarray(wt[72:])
    jj = np.arange(128, dtype=np.int32).reshape(128, 1)
    maps = []
    for b in range(B):
        flat = np.zeros((16, 128 * 128 + 384), dtype=np.float16)
        flat[:, :128 * 128] = data[b].astype(np.float16).reshape(16, -1)
        # imcol[(dy*3+dx)*16+c, h*128+w] = data[c, h+dy, w+dx]
        imcol = np.empty((144, RAST), dtype=np.float16)
        for t in range(9):
            dy, dx = divmod(t, 3)
            off = dy * 128 + dx
            imcol[t * 16:(t + 1) * 16] = flat[:, off:off + RAST]
        idxr = np.full((126, 128), 255, dtype=np.int32)
        idxr[:, :126] = kernel_idx[b].astype(np.int32)
        ohb = (idxr.reshape(1, RAST) == jj).astype(ml_dtypes.float8_e4m3)
        maps.append({
            "bufA": np.ascontiguousarray(imcol[:72]),
            "bufB": np.ascontiguousarray(imcol[72:]),
            "oh": ohb,
            "w8": w8,
            "w1": w1,
        })
    return maps


def kernel(data, kernel_idx, weights, _trace=False):
    from concourse.bass_utils import run_bass_kernel_spmd

    data = np.asarray(data, dtype=np.float32)
    kernel_idx = np.asarray(kernel_idx)
    weights = np.asarray(weights, dtype=np.float32)
    B = data.shape[0]
    nc = _get_nc()
    res = run_bass_kernel_spmd(nc, _in_maps(data, kernel_idx, weights),
                               core_ids=list(range(B)), trace=_trace)
    out = np.stack([
        r["o"].reshape(16384)[:RAST].reshape(126, 128)[:, :126]
        for r in res.results
    ])
    if _trace:
        return out.astype(np.float32), res
    return out.astype(np.float32)
